# revision 7
# baseline (speedup 1.0000x reference)
"""Distributed Trainium2 kernel for the ADMM-NN fixed-point iteration:

    for _ in range(N):
        x = W @ x + b
        x[idx1:idx2] = clip(x[idx1:idx2], l, u)

v2-opt: 4-way column-tiled TensorE GEMV, one bf16 AllGather per iteration.

  - Row-shard W: core i owns 1024 rows, permuted so local y layout is
    j = g*256 + n with group g in {0..3}; group 3 is the clamp segment.
  - W resident in SBUF as bf16 [128, 64*4*256] (16 MB/core).
  - Per k-tile t: 4 concurrent matmuls (tile_position=(0,32g)), each
    [128,1] x-column stationary x [128,256] W moving -> psum row 32g.
    4 independent moving streams ~= 4x the single-stream W bandwidth.
  - Bias is folded into the accumulation as a 65th slot (stationary =
    e0, moving = bias rows), so the post-matmul chain is just a DVE
    bf16 convert + clamp.
  - AllGather (bf16, 2KB/core in, 16KB out); the gathered [128,64]
    partition-major layout IS the stationary x layout by construction
    of the host-side W column permutation.  The x tile is split in two
    [128,32] halves so the next burst can start as soon as the first
    half of the gather output has landed.

kernel(**inputs) takes FULL unsharded inputs, returns the FULL output.
"""

import numpy as np
import ml_dtypes

NCORES = 8
D = 8192
ROWS = D // NCORES  # 1024
NT = 64             # k-tiles of 128
HT = NT // 2
P = 128
NG = 4              # column-tile groups
GW = ROWS // NG     # 256 outputs per group
NWCH = 16

_nc_cache = {}


def _perm(idx1, idx2):
    """perm[i*1024 + j] = global row owned by core i at local position j,
    local layout j = g*256 + n, group 3 = clamp rows."""
    assert idx2 == D and idx1 == 6144
    un = ROWS - (idx2 - idx1) // NCORES  # 768
    parts = []
    for i in range(NCORES):
        parts.append(np.arange(un * i, un * (i + 1)))
        parts.append(idx1 + np.arange(256 * i, 256 * (i + 1)))
    return np.concatenate(parts), un


def _colmap():
    """colmap[p, t] = flat local-order index (core-major) of the x value at
    stationary cell (p, t): AllGather output [128, 64] partition-major."""
    p = np.arange(P)[:, None]
    t = np.arange(NT)[None, :]
    return p * NT + t


def _build_nc(n_iter, l_val, u_val):
    import concourse.bacc as bacc
    import concourse.mybir as mybir
    from concourse import tile

    nc = bacc.Bacc(None, target_bir_lowering=False, num_devices=NCORES)
    wcols = NT * NG * GW  # 65536
    w_ext = [
        nc.declare_dram_parameter(
            f"W{c}", [P, wcols // NWCH], mybir.dt.bfloat16, isOutput=False
        )
        for c in range(NWCH)
    ]
    bw_ext = nc.declare_dram_parameter("biasw", [P, ROWS], mybir.dt.float32, isOutput=False)
    x0_ext = nc.declare_dram_parameter("x0", [P, NT], mybir.dt.float32, isOutput=False)
    out_ext = nc.declare_dram_parameter("out", [1, ROWS], mybir.dt.float32, isOutput=True)

    with tile.TileContext(nc) as tc:
        with (
            tc.tile_pool(name="wpool", bufs=1) as wpool,
            tc.tile_pool(name="cpool", bufs=1) as cpool,
            tc.tile_pool(name="xpool", bufs=2) as xpool,
            tc.tile_pool(name="ypool", bufs=2) as ypool,
            tc.tile_pool(name="ps", bufs=2, space="PSUM") as pspool,
            tc.tile_pool(name="dram", bufs=2, space="DRAM") as dpool,
        ):
            wt = []
            for c in range(NWCH):
                w = wpool.tile([P, wcols // NWCH], mybir.dt.bfloat16, tag=f"W{c}")
                nc.sync.dma_start(w[:], w_ext[c][:])
                wt.append(w)
            bw = cpool.tile([P, ROWS], mybir.dt.float32, tag="bw")
            nc.sync.dma_start(bw[:], bw_ext[:])
            ones = cpool.tile([P, 1], mybir.dt.float32, tag="ones")
            nc.vector.memset(ones[:], 0.0)
            nc.vector.memset(ones[0:1, :], 1.0)

            xf = cpool.tile([P, NT], mybir.dt.float32, tag="xf")
            nc.sync.dma_start(xf[:], x0_ext[:])
            xbA = xpool.tile([P, HT], mybir.dt.bfloat16, tag="xbA")
            xbB = xpool.tile([P, HT], mybir.dt.bfloat16, tag="xbB")
            nc.vector.tensor_copy(xbA[:], xf[:, 0:HT])
            nc.vector.tensor_copy(xbB[:], xf[:, HT:NT])

            def wblock(t, g):
                col = (t * NG + g) * GW
                c, off = divmod(col, wcols // NWCH)
                return wt[c][:, off : off + GW]

            for k in range(n_iter):
                last = k == n_iter - 1
                ps = pspool.tile([P, GW], mybir.dt.float32, tag="ps")
                for t in range(NT):
                    xb = xbA if t < HT else xbB
                    tt = t if t < HT else t - HT
                    for g in range(NG):
                        nc.tensor.matmul(
                            ps[32 * g : 32 * g + 1, :],
                            xb[:, tt : tt + 1],
                            wblock(t, g),
                            start=(t == 0),
                            stop=False,
                            tile_position=(0, 32 * g),
                        )
                for g in range(NG):  # bias slot
                    nc.tensor.matmul(
                        ps[32 * g : 32 * g + 1, :],
                        ones[:, 0:1],
                        bw[:, g * GW : (g + 1) * GW],
                        start=False,
                        stop=True,
                        tile_position=(0, 32 * g),
                    )
                if last:
                    yf = ypool.tile([P, GW], mybir.dt.float32, tag="yf")
                    nc.vector.tensor_copy(yf[:, :], ps[:, :])
                    nc.vector.tensor_scalar(
                        yf[96:97, :], yf[96:97, :], float(l_val), float(u_val),
                        mybir.AluOpType.max, mybir.AluOpType.min,
                    )
                    for g in range(NG):
                        nc.sync.dma_start(
                            out_ext[:, g * GW : (g + 1) * GW], yf[32 * g : 32 * g + 1, :]
                        )
                else:
                    yb = ypool.tile([P, GW], mybir.dt.bfloat16, tag="yb")
                    nc.vector.tensor_copy(yb[:, :], ps[:, :])
                    nc.vector.tensor_scalar(
                        yb[96:97, :], yb[96:97, :], float(l_val), float(u_val),
                        mybir.AluOpType.max, mybir.AluOpType.min,
                    )
                    agin = dpool.tile([1, ROWS], mybir.dt.bfloat16, tag="agin")
                    for g in range(NG):
                        nc.sync.dma_start(
                            agin[:, g * GW : (g + 1) * GW], yb[32 * g : 32 * g + 1, :]
                        )
                    agout = dpool.tile([P, NT], mybir.dt.bfloat16, tag="agout")
                    nc.gpsimd.collective_compute(
                        "AllGather",
                        mybir.AluOpType.bypass,
                        replica_groups=[list(range(NCORES))],
                        ins=[agin.opt()],
                        outs=[agout.opt()],
                    )
                    xbA = xpool.tile([P, HT], mybir.dt.bfloat16, tag="xbA")
                    xbB = xpool.tile([P, HT], mybir.dt.bfloat16, tag="xbB")
                    nc.sync.dma_start(xbA[:], agout[:, 0:HT])
                    nc.sync.dma_start(xbB[:], agout[:, HT:NT])
    nc.compile()
    return nc


def _get_nc(n_iter, l_val, u_val):
    key = (n_iter, float(l_val), float(u_val))
    if key not in _nc_cache:
        _nc_cache[key] = _build_nc(n_iter, l_val, u_val)
    return _nc_cache[key]


def _prep_in_maps(x, W, b, idx1, idx2):
    perm, _un = _perm(idx1, idx2)
    cm = _colmap()
    colidx = perm[cm.reshape(-1)].reshape(P, NT)
    bf16 = ml_dtypes.bfloat16
    xp = np.asarray(x, np.float32)
    x0_layout = np.ascontiguousarray(xp[colidx], np.float32)
    in_maps = []
    for i in range(NCORES):
        rows_i = perm[ROWS * i : ROWS * (i + 1)]
        Wi = W[rows_i]
        Wc = Wi[:, colidx.reshape(-1)].reshape(ROWS, P, NT)
        Wt = np.ascontiguousarray(
            np.transpose(Wc, (1, 2, 0)).reshape(P, NT * ROWS)
        ).astype(bf16)
        m = {
            f"W{c}": np.ascontiguousarray(
                Wt[:, c * (NT * ROWS // NWCH) : (c + 1) * (NT * ROWS // NWCH)]
            )
            for c in range(NWCH)
        }
        m["x0"] = x0_layout
        bl = np.asarray(b, np.float32)[rows_i]
        bmat = np.zeros((P, ROWS), np.float32)
        bmat[0, :] = bl
        m["biasw"] = bmat
        in_maps.append(m)
    return in_maps, perm


def run(x, W, b, l, u, idx1, idx2, N, trace=False, trace_kwargs=None):
    from concourse.bass_utils import run_bass_kernel_spmd

    x = np.asarray(x, np.float32)
    W = np.asarray(W, np.float32)
    b = np.asarray(b, np.float32)
    l = float(np.asarray(l))
    u = float(np.asarray(u))
    idx1 = int(np.asarray(idx1))
    idx2 = int(np.asarray(idx2))
    N = int(np.asarray(N))
    assert x.shape == (D,) and W.shape == (D, D) and b.shape == (D,)
    assert N >= 1

    nc = _get_nc(N, l, u)
    in_maps, perm = _prep_in_maps(x, W, b, idx1, idx2)
    res = run_bass_kernel_spmd(
        nc,
        in_maps,
        core_ids=list(range(NCORES)),
        trace=trace,
        **(trace_kwargs or {}),
    )
    chunks = [np.asarray(res.results[i]["out"], np.float32).reshape(ROWS) for i in range(NCORES)]
    out = np.empty(D, np.float32)
    out[perm] = np.concatenate(chunks)
    return out, res


def kernel(**inputs):
    out, _ = run(
        inputs["x"],
        inputs["W"],
        inputs["b"],
        inputs["l"],
        inputs["u"],
        inputs["idx1"],
        inputs["idx2"],
        inputs["N"],
        trace=False,
    )
    return out
